# revision 1
# baseline (speedup 1.0000x reference)
"""TRN2 Bass kernel for nn_LinearAttention (B=4, L=4096, D=1024, H=16, dh=64).

Strategy: shard (batch, head-group) across 8 cores — core c handles batch c//2,
heads (c%2)*8..(c%2)*8+8. Zero cross-core communication; the two half-head
partial output projections per batch are summed on the host during unshard.

Per-core pipeline (super-tiles of 512 tokens, scan chunks of 128):
  1. QKV projection (f32r matmuls, full-rate at moving dim 512): q,k produced
     transposed (feat, tok) with phi=elu+1 fused on the PSUM->SBUF path
     (phi = relu(x) + exp(min(x,0)), exact); v produced (tok, feat) with a
     ones-column appended so the denominator rides along in the matmuls.
  2. Chunked linear attention: per head-chunk A^T = K^T Q (bf16, masked
     causal-inclusive), O = A^T_m^T V' + Q S'_prev (second matmul fp32),
     state S' += K^T V' kept in full fp32 (S' holds [S | z]).
     den = max(O[:,64], eps), out = O[:,:64] * recip(den).
     PSUM tiles are shared across heads with a single accumulation group per
     bank (start=True only on the bank's first matmul) - interleaved
     accumulation groups in one bank crash on HW.
  3. PE-transpose per-chunk outputs to (feat, tok), output projection with
     f32r matmuls, bounce PSUM->SBUF, DMA the transposed partial to HBM.
Dtype notes: f32r ~1.9e-4 matmul rel err (full rate at N>=256), bf16 ~3e-3;
scan state stays fp32 so error does not accumulate over the 32-chunk scan.
End-to-end rel err vs fp32 reference: 1.8e-3.
"""
import sys

sys.path.insert(0, "/opt/trn_rl_repo")
import numpy as np

D = 1024
L = 4096
B = 4
H = 16
DH = 64
FPC = 512          # features per core (8 heads x 64)
C = 128            # scan chunk
ST = 512           # super-tile tokens
NCH = ST // C      # 4
NST = L // ST      # 8
EPS = 1e-6

_CACHE = {}
import os
ATT_BF16 = os.environ.get("ATT_BF16", "1") == "1"


def _build_nc(att_bf16=ATT_BF16):
    import concourse.bacc as bacc
    import concourse.mybir as mybir
    import concourse.tile as tile

    dt = mybir.dt
    f32, f32r = dt.float32, dt.float32r
    adt = dt.bfloat16 if att_bf16 else f32
    Alu = mybir.AluOpType
    Act = mybir.ActivationFunctionType

    nc = bacc.Bacc("TRN2", target_bir_lowering=False, debug=True)

    xT_d = nc.dram_tensor("xT", [D, L], f32r, kind="ExternalInput")
    w1qk_d = nc.dram_tensor("w1qk", [D, 2 * FPC], f32r, kind="ExternalInput")
    w1v_d = nc.dram_tensor("w1v", [D, FPC], f32r, kind="ExternalInput")
    w2_d = nc.dram_tensor("w2", [FPC, D], f32r, kind="ExternalInput")
    maskT_d = nc.dram_tensor("maskT", [C, C], f32, kind="ExternalInput")
    id128_d = nc.dram_tensor("id128", [128, 128], f32, kind="ExternalInput")
    id64_d = nc.dram_tensor("id64", [128, DH], f32, kind="ExternalInput")
    outT_d = nc.dram_tensor("outT", [D, L], f32, kind="ExternalOutput")

    with tile.TileContext(nc) as tc:
        with (
            tc.tile_pool(name="wp", bufs=1) as wp,
            tc.tile_pool(name="xp", bufs=2) as xp,
            tc.tile_pool(name="qp", bufs=2) as qp,
            tc.tile_pool(name="vp", bufs=2) as vp,
            tc.tile_pool(name="scr", bufs=3) as scr,
            tc.tile_pool(name="atsp", bufs=4) as atsp,
            tc.tile_pool(name="kp", bufs=4) as kp,
            tc.tile_pool(name="dp", bufs=8) as dp,
            tc.tile_pool(name="op", bufs=6) as op,
            tc.tile_pool(name="osp", bufs=2) as osp,
            tc.tile_pool(name="szp", bufs=1) as szp,
            tc.tile_pool(name="projp", bufs=2, space="PSUM") as projp,
            tc.tile_pool(name="atp", bufs=2, space="PSUM") as atp,
            tc.tile_pool(name="opp", bufs=2, space="PSUM") as opp,
            tc.tile_pool(name="miscp", bufs=1, space="PSUM") as miscp,
            tc.tile_pool(name="szkvp", bufs=1, space="PSUM") as szkvp,
        ):
            # ---- resident constants (spread across DMA issuers; xT st0 goes first) ----
            w1qk = wp.tile([128, 8, 2 * FPC], f32r, tag="w1qk")
            for cc in range(2):
                nc.sync.dma_start(
                    w1qk[:, 4 * cc : 4 * cc + 4, :],
                    w1qk_d[:].rearrange("(c p) f -> p c f", p=128)[
                        :, 4 * cc : 4 * cc + 4, :
                    ],
                )
            w1v = wp.tile([128, 8, FPC], f32r, tag="w1v")
            nc.scalar.dma_start(w1v[:], w1v_d[:].rearrange("(c p) f -> p c f", p=128))
            w2 = wp.tile([128, 4, D], f32r, tag="w2")
            nc.scalar.dma_start(w2[:], w2_d[:].rearrange("(g p) f -> p g f", p=128))
            maskT = wp.tile([C, C], f32, tag="maskT")
            nc.gpsimd.dma_start(maskT[:], maskT_d[:])
            id128 = wp.tile([128, 128], f32, tag="id128")
            nc.gpsimd.dma_start(id128[:], id128_d[:])
            id64 = wp.tile([128, DH], f32, tag="id64")
            nc.gpsimd.dma_start(id64[:], id64_d[:])
            id64a = wp.tile([128, DH], adt, tag="id64a")
            nc.vector.tensor_copy(id64a[:], id64[:])
            id128a = wp.tile([128, 128], adt, tag="id128a")
            nc.vector.tensor_copy(id128a[:], id128[:])
            id128r = wp.tile([128, 128], f32r, tag="id128r")
            nc.vector.tensor_copy(id128r[:], id128[:])

            # persistent scan state: [S | z] per head, packed 2 heads/partition-half
            Sz = szp.tile([128, 4, DH + 1], f32, tag="Sz")
            nc.vector.memset(Sz[:], 0.0)

            for st in range(NST):
                t0 = st * ST
                xT = xp.tile([128, 8, ST], f32r, tag="xT")
                xsrc = xT_d[:, t0 : t0 + ST].rearrange("(c p) t -> p c t", p=128)
                nc.sync.dma_start(xT[:, 0:4, :], xsrc[:, 0:4, :])
                nc.sync.dma_start(xT[:, 4:8, :], xsrc[:, 4:8, :])

                # ---- stage 1a: q,k transposed (feat, tok) with phi fused ----
                qT32 = qp.tile([128, 4, ST], f32, tag="qT32")
                if att_bf16:
                    qTb = qp.tile([128, 4, ST], adt, tag="qTb")
                else:
                    qTb = qT32
                kTb = qp.tile([128, 4, ST], adt, tag="kTb")
                for fc in range(8):
                    pq = projp.tile([128, ST], f32, tag="proj")
                    for dc in range(8):
                        nc.tensor.matmul(
                            pq[:],
                            w1qk[:, dc, fc * 128 : (fc + 1) * 128],
                            xT[:, dc, :],
                            start=(dc == 0),
                            stop=(dc == 7),
                        )
                    tneg = scr.tile([128, ST], f32, tag="tneg")
                    nc.scalar.activation(tneg[:], pq[:], Act.Relu, scale=-1.0)
                    texp = scr.tile([128, ST], f32, tag="texp")
                    nc.scalar.activation(texp[:], tneg[:], Act.Exp, scale=-1.0)
                    # phi(x) = relu(x) + exp(min(x,0))
                    if fc < 4:
                        nc.vector.scalar_tensor_tensor(
                            qT32[:, fc, :], pq[:], 0.0, texp[:], Alu.max, Alu.add
                        )
                        if att_bf16:
                            nc.vector.tensor_copy(qTb[:, fc, :], qT32[:, fc, :])
                    else:
                        nc.vector.scalar_tensor_tensor(
                            kTb[:, fc - 4, :], pq[:], 0.0, texp[:], Alu.max, Alu.add
                        )

                # ---- stage 1b: v in (tok, feat) + ones column ----
                v1 = vp.tile([128, NCH, 8, DH + 1], adt, tag="v1")
                nc.vector.memset(v1[:, :, :, DH], 1.0)
                for tcc in range(NCH):
                    pv = projp.tile([128, FPC], f32, tag="proj")
                    for dc in range(8):
                        nc.tensor.matmul(
                            pv[:],
                            xT[:, dc, tcc * 128 : (tcc + 1) * 128],
                            w1v[:, dc, :],
                            start=(dc == 0),
                            stop=(dc == 7),
                        )
                    nc.scalar.copy(
                        v1[:, tcc, :, 0:DH], pv[:].rearrange("p (h e) -> p h e", e=DH)
                    )

                # ---- stage 2: chunked linear attention scan ----
                outT_sb = osp.tile([128, 4, ST], f32r, tag="outT")
                out_cs = []
                for tcc in range(NCH):
                    out_c = op.tile([128, 8, DH], f32r, tag="out_c")
                    out_cs.append(out_c)

                    # k transposed to (tok, dh): 4 pair-transposes share one
                    # PSUM bank (single accumulation group), one bulk copy out
                    ksb = kp.tile([128, 8, DH], adt, tag="ksb")
                    ktr = miscp.tile([128, 4, 128], adt, tag="misc")
                    for j in range(4):
                        kTj = kTb[:, j, tcc * 128 : (tcc + 1) * 128]
                        nc.tensor.matmul(
                            ktr[:, j, :], kTj, id128a[:],
                            is_transpose=True,
                            start=(j == 0), stop=(j == 3),
                            skip_group_check=True,
                        )
                    nc.scalar.copy(
                        ksb[:], ktr[:].rearrange("p j (h e) -> p (j h) e", e=DH)
                    )

                    # per-head: A^T = K^T Q, mask, O = A^T_m^T V' + Q S'
                    # 4 heads share one PSUM bank; ONE accumulation group per bank
                    # (start=True only on the bank's first matmul - per-element
                    # has_written gives overwrite-on-first-touch for later slices)
                    Og = []
                    for j in range(2):
                        Oj = opp.tile([128, 4 * (DH + 1)], f32, tag="O4")
                        Og.append(Oj)
                    for h in range(8):
                        po = 64 * (h % 2)
                        fq = h // 2
                        qTh32 = qT32[po : po + 64, fq, tcc * 128 : (tcc + 1) * 128]
                        qThb = qTb[po : po + 64, fq, tcc * 128 : (tcc + 1) * 128]
                        kTh = kTb[po : po + 64, fq, tcc * 128 : (tcc + 1) * 128]
                        v1h = v1[:, tcc, h, :]
                        Szh = Sz[po : po + 64, h // 2, :]

                        ATp = atp.tile([C, C], f32, tag="ATp")
                        nc.tensor.matmul(ATp[:], kTh, qThb, start=True, stop=True)
                        ATs = atsp.tile([C, C], adt, tag="ATs")
                        nc.vector.tensor_tensor(ATs[:], ATp[:], maskT[:], Alu.mult)

                        Oh = Og[h // 4][:, (h % 4) * (DH + 1) : (h % 4 + 1) * (DH + 1)]
                        nc.tensor.matmul(
                            Oh, ATs[:], v1h,
                            start=(h % 4 == 0), stop=False,
                            skip_group_check=True,
                        )
                        nc.tensor.matmul(
                            Oh, qTh32, Szh,
                            start=False, stop=(h % 4 == 3),
                            skip_group_check=True,
                        )

                    # state update: head pairs stacked in one PSUM tile
                    for j in range(4):
                        Szkv = szkvp.tile([128, DH + 1], f32, tag="szkv")
                        nc.tensor.matmul(
                            Szkv[0:64, :], ksb[:, 2 * j, :], v1[:, tcc, 2 * j, :],
                            start=True, stop=False, skip_group_check=True,
                        )
                        nc.tensor.matmul(
                            Szkv[64:128, :], ksb[:, 2 * j + 1, :],
                            v1[:, tcc, 2 * j + 1, :],
                            start=True, stop=True, skip_group_check=True,
                        )
                        nc.vector.tensor_tensor(
                            Sz[:, j, :], Sz[:, j, :], Szkv[:], Alu.add
                        )

                    # batched normalization: den/recip/div for 4 heads at once
                    for j in range(2):
                        Ov = Og[j][:].rearrange("p (h e) -> p h e", e=DH + 1)
                        den = dp.tile([C, 4], f32, tag="den")
                        nc.vector.tensor_scalar(
                            den[:], Ov[:, :, DH], EPS, None, Alu.max
                        )
                        rden = dp.tile([C, 4], f32, tag="rden")
                        nc.vector.reciprocal(rden[:], den[:])
                        nc.vector.tensor_tensor(
                            out_c[:, 4 * j : 4 * j + 4, :],
                            Ov[:, :, 0:DH],
                            rden[:].unsqueeze(2).broadcast_to([C, 4, DH]),
                            Alu.mult,
                        )

                # transpose chunk outputs to (feat, tok) at super-tile end
                for tcc in range(NCH):
                    oc_flat = out_cs[tcc][:].rearrange("p h e -> p (h e)")
                    trp = miscp.tile([128, 4, C], f32r, tag="misc")
                    for g in range(4):
                        nc.tensor.matmul(
                            trp[:, g, :], oc_flat[:, g * 128 : (g + 1) * 128],
                            id128r[:],
                            is_transpose=True,
                            start=(g == 0), stop=(g == 3),
                            skip_group_check=True,
                        )
                    nc.scalar.copy(
                        outT_sb[:, :, tcc * 128 : (tcc + 1) * 128], trp[:]
                    )

                # ---- stage 3: output projection (partial, transposed) ----
                for oc in range(8):
                    po3 = projp.tile([128, ST], f32, tag="proj")
                    for g in range(4):
                        nc.tensor.matmul(
                            po3[:],
                            w2[:, g, oc * 128 : (oc + 1) * 128],
                            outT_sb[:, g, :],
                            start=(g == 0),
                            stop=(g == 3),
                        )
                    o3sb = scr.tile([128, ST], f32, tag="o3sb")
                    nc.scalar.copy(o3sb[:], po3[:])
                    nc.sync.dma_start(
                        outT_d[oc * 128 : (oc + 1) * 128, t0 : t0 + ST], o3sb[:]
                    )

    nc.finalize()
    return nc


def _get_nc():
    key = ("nc", ATT_BF16)
    if key not in _CACHE:
        _CACHE[key] = _build_nc(ATT_BF16)
    return _CACHE[key]


def _make_in_maps(x, w_qkv, w_out):
    maskT = np.triu(np.ones((C, C), np.float32))  # maskT[j,i] = 1 if j <= i
    id128 = np.eye(128, dtype=np.float32)
    id64 = np.tile(np.eye(DH, dtype=np.float32), (2, 1))
    in_maps = []
    for core in range(8):
        b, g = core // 2, core % 2
        xT = np.ascontiguousarray(x[b].T)
        w1qk = np.ascontiguousarray(
            np.concatenate(
                [
                    w_qkv[512 * g : 512 * (g + 1)],
                    w_qkv[1024 + 512 * g : 1024 + 512 * (g + 1)],
                ],
                axis=0,
            ).T
        )
        w1v = np.ascontiguousarray(w_qkv[2048 + 512 * g : 2048 + 512 * (g + 1)].T)
        w2 = np.ascontiguousarray(w_out[:, 512 * g : 512 * (g + 1)].T)
        in_maps.append(
            {
                "xT": xT,
                "w1qk": w1qk,
                "w1v": w1v,
                "w2": w2,
                "maskT": maskT,
                "id128": id128,
                "id64": id64,
            }
        )
    return in_maps


def _run(inputs, trace=False):
    from concourse.bass_utils import run_bass_kernel_spmd

    nc = _get_nc()
    in_maps = _make_in_maps(inputs["x"], inputs["w_qkv"], inputs["w_out"])
    res = run_bass_kernel_spmd(nc, in_maps, core_ids=list(range(8)), trace=trace)
    out = np.empty((B, L, D), np.float32)
    for b in range(B):
        p0 = res.results[2 * b]["outT"]
        p1 = res.results[2 * b + 1]["outT"]
        out[b] = (p0 + p1).T
    return out, res


def kernel(x, w_qkv, w_out):
    out, _ = _run({"x": x, "w_qkv": w_qkv, "w_out": w_out})
    return out



# revision 16
# speedup vs baseline: 1.1298x; 1.1298x over previous
"""TRN2 Bass kernel for nn_LinearAttention (B=4, L=4096, D=1024, H=16, dh=64).

Sharding: core c handles batch c//2, heads (c%2)*8..+8. Zero cross-core
communication; the two half-head partial output projections per batch are
summed on the host during unshard (partials DMA'd in bf16).

Per-core pipeline (super-tiles of 512 tokens, scan chunks of 128), with the
projections in fp8 DoubleRow (0.5 cycles/row, 256-contraction per matmul):
  - q/k proj: x and w both plain e4m3 (w pre-scaled by 2^6 to clear the
    e4m3 subnormal range).  Scaled-phi trick: phi_s = max(pq,0) + 64*exp(
    min(pq/64,0)) = 64*phi(x); the 2^6 per-side scale cancels in num/den
    (EPS scaled by 2^12).  q/k quantization error is damped by the
    normalization ratio (measured end-to-end ~2.8e-3).
  - v proj: error-compensated fp8: x = x_hi + x_lo (two e4m3 planes,
    host-split), w row-pairs (w_hi, w_lo): pv = (x_hi+x_lo)W_hi + x_hi W_lo
    (12 DoubleRow matmuls per 128-token chunk).  v1 copy descales by 2^-6.
  - attention: bf16 chunked scan, A^T = K^T Q masked (4 heads batched per
    PSUM bank, one DVE mask op per 4 heads), O = A^T_m^T V' + Q S'_b.
    State S' accumulates IN PSUM across all 32 chunks (matmul accumulation,
    no DVE adds); a bf16 copy S'_b is taken each chunk for the Q S' matmul
    (moving operand must be non-f32 so the matmul runs at 1 cycle/row).
  - out proj: w2 f32r stationary x bf16 moving (1 cycle/row), bf16 partial
    written to HBM (halves output DMA traffic).
Software-pipelined emission: projection of super-tile st interleaves with
attention/output of st-1 so the PE is never hostage to the phi (Act) chain.
End-to-end rel err vs fp32 reference: ~2.8e-3.
"""
import sys

sys.path.insert(0, "/opt/trn_rl_repo")
import numpy as np
import ml_dtypes

D = 1024
L = 4096
B = 4
H = 16
DH = 64
FPC = 512          # features per core (8 heads x 64)
C = 128            # scan chunk
ST = 512           # super-tile tokens
NCH = ST // C      # 4
NST = L // ST      # 8
S6 = 64.0          # 2^6 weight pre-scale
EPS_S = 1e-6 * S6 * S6
LN_S6 = float(np.log(S6))

_CACHE = {}
import os
USE_FP8 = os.environ.get("K_FP8", "1") == "1"
USE_SZPSUM = os.environ.get("K_SZPSUM", "1") == "1"
ODMA_GP = os.environ.get("K_ODMA", "gp") == "gp"


def _build_nc(use_fp8=None, use_szpsum=None):
    use_fp8 = USE_FP8 if use_fp8 is None else use_fp8
    use_szpsum = USE_SZPSUM if use_szpsum is None else use_szpsum
    odma_gp = ODMA_GP
    import concourse.bacc as bacc
    import concourse.mybir as mybir
    import concourse.tile as tile

    dt = mybir.dt
    f32, f32r, bf16, f8 = dt.float32, dt.float32r, dt.bfloat16, dt.float8e4
    Alu = mybir.AluOpType
    Act = mybir.ActivationFunctionType
    DR = mybir.MatmulPerfMode.DoubleRow

    nc = bacc.Bacc("TRN2", target_bir_lowering=False, debug=True)

    if use_fp8:
        xhi_d = nc.dram_tensor("xhi", [D, L], f8, kind="ExternalInput")
        xlo_d = nc.dram_tensor("xlo", [D, L], f8, kind="ExternalInput")
        w8qk_d = nc.dram_tensor("w8qk", [D, 2 * FPC], f8, kind="ExternalInput")
        w8v_d = nc.dram_tensor("w8v", [D, 2, FPC], f8, kind="ExternalInput")
        w8vlo_d = nc.dram_tensor("w8vlo", [D, FPC], f8, kind="ExternalInput")
    else:
        xT_d = nc.dram_tensor("xT", [D, L], f32r, kind="ExternalInput")
        w1qk_d = nc.dram_tensor("w1qk", [D, 2 * FPC], f32r, kind="ExternalInput")
        w1v_d = nc.dram_tensor("w1v", [D, FPC], f32r, kind="ExternalInput")
    w2_d = nc.dram_tensor("w2", [FPC, D], bf16, kind="ExternalInput")
    maskb_d = nc.dram_tensor("maskb", [C, 4, C], bf16, kind="ExternalInput")
    id128b_d = nc.dram_tensor("id128b", [128, 128], bf16, kind="ExternalInput")
    outT_d = nc.dram_tensor("outT", [D, L], bf16, kind="ExternalOutput")

    with tile.TileContext(nc) as tc:
        with (
            tc.tile_pool(name="wp", bufs=1) as wp,
            tc.tile_pool(name="xp", bufs=2) as xp,
            tc.tile_pool(name="qp", bufs=2) as qp,
            tc.tile_pool(name="vp", bufs=2) as vp,
            tc.tile_pool(name="scr", bufs=3) as scr,
            tc.tile_pool(name="atsp", bufs=3) as atsp,
            tc.tile_pool(name="kp", bufs=2) as kp,
            tc.tile_pool(name="dp", bufs=4) as dp,
            tc.tile_pool(name="op", bufs=5) as op,
            tc.tile_pool(name="osp", bufs=2) as osp,
            tc.tile_pool(name="o3p", bufs=2) as o3p,
            tc.tile_pool(name="szbp", bufs=1) as szbp,
            tc.tile_pool(name="projp", bufs=2, space="PSUM") as projp,
            tc.tile_pool(name="atp", bufs=2, space="PSUM") as atp,
            tc.tile_pool(name="opp", bufs=2, space="PSUM") as opp,
            tc.tile_pool(name="miscp", bufs=1, space="PSUM") as miscp,
            tc.tile_pool(name="szp", bufs=1, space="PSUM") as szp,
            tc.tile_pool(name="szsb", bufs=1) as szsb,
        ):
            # ---- resident weights/constants, spread across DMA queues ----
            if use_fp8:
                w8qk = wp.tile([128, 8, 2 * FPC], f8, tag="w8qk")
                wqk_src = w8qk_d[:].rearrange("(c p) f -> p c f", p=128)
                nc.sync.dma_start(w8qk[:, :, 0:FPC], wqk_src[:, :, 0:FPC])
                xhi0 = xp.tile([128, 2, 8, ST], f8, tag="xT8")
                nc.scalar.dma_start(
                    xhi0[:, 0], xhi_d[:, 0:ST].rearrange("(c p) t -> p c t", p=128)
                )
                nc.sync.dma_start(
                    w8qk[:, :, FPC : 2 * FPC], wqk_src[:, :, FPC : 2 * FPC]
                )
                nc.scalar.dma_start(
                    xhi0[:, 1], xlo_d[:, 0:ST].rearrange("(c p) t -> p c t", p=128)
                )
                w8v = wp.tile([128, 8, 2, FPC], f8, tag="w8v")
                nc.scalar.dma_start(
                    w8v[:], w8v_d[:].rearrange("(c p) two f -> p c two f", p=128)
                )
                w8vlo = wp.tile([128, 4, 2, FPC], f8, tag="w8vlo")
                nc.scalar.dma_start(
                    w8vlo[:],
                    w8vlo_d[:].rearrange("(dcp two p) f -> p dcp two f", p=128, two=2),
                )
            else:
                w1qk = wp.tile([128, 8, 2 * FPC], f32r, tag="w1qk")
                wqk_src = w1qk_d[:].rearrange("(c p) f -> p c f", p=128)
                nc.sync.dma_start(w1qk[:, :, 0:FPC], wqk_src[:, :, 0:FPC])
                xhi0 = xp.tile([128, 8, ST], f32r, tag="xT8")
                nc.scalar.dma_start(
                    xhi0[:, 0:4, :],
                    xT_d[:, 0:ST].rearrange("(c p) t -> p c t", p=128)[:, 0:4, :],
                )
                nc.sync.dma_start(
                    w1qk[:, :, FPC : 2 * FPC], wqk_src[:, :, FPC : 2 * FPC]
                )
                nc.scalar.dma_start(
                    xhi0[:, 4:8, :],
                    xT_d[:, 0:ST].rearrange("(c p) t -> p c t", p=128)[:, 4:8, :],
                )
                w1v = wp.tile([128, 8, FPC], f32r, tag="w1v")
                nc.scalar.dma_start(
                    w1v[:], w1v_d[:].rearrange("(c p) f -> p c f", p=128)
                )
            maskb = wp.tile([C, 4, C], bf16, tag="maskb")
            nc.gpsimd.dma_start(maskb[:], maskb_d[:])
            id128b = wp.tile([128, 128], bf16, tag="id128b")
            nc.gpsimd.dma_start(id128b[:], id128b_d[:])
            w2 = wp.tile([128, 4, D], bf16, tag="w2")
            nc.gpsimd.dma_start(w2[:], w2_d[:].rearrange("(g p) f -> p g f", p=128))

            biasln = wp.tile([128, 1], f32, tag="biasln")
            nc.vector.memset(biasln[:], LN_S6)

            # persistent scan state: accumulated in PSUM across all chunks
            if use_szpsum:
                # padded to 2048B/partition so [64:128] slice offsets stay
                # aligned with the executor's 2KB pending-zero rows
                SzT = szp.tile([128, 4, 2 * DH], f32, tag="Sz", name="Sz")
                SzW = 2 * DH
            else:
                SzT = szsb.tile([128, 4, DH + 1], f32, tag="Sz", name="Sz")
                SzW = DH + 1
                nc.vector.memset(SzT[:], 0.0)
            # bf16 state snapshot (moving operand of the Q.S matmul)
            Szb = szbp.tile([128, 4, DH + 1], bf16, tag="Szb")
            nc.vector.memset(Szb[:], 0.0)

            state = {}

            def emit_xdma(st):
                t0 = st * ST
                if st == 0:
                    xT8 = xhi0
                elif use_fp8:
                    xT8 = xp.tile([128, 2, 8, ST], f8, tag="xT8")
                    nc.sync.dma_start(
                        xT8[:, 0],
                        xhi_d[:, t0 : t0 + ST].rearrange("(c p) t -> p c t", p=128),
                    )
                    nc.sync.dma_start(
                        xT8[:, 1],
                        xlo_d[:, t0 : t0 + ST].rearrange("(c p) t -> p c t", p=128),
                    )
                else:
                    xT8 = xp.tile([128, 8, ST], f32r, tag="xT8")
                    xsrc = xT_d[:, t0 : t0 + ST].rearrange("(c p) t -> p c t", p=128)
                    nc.sync.dma_start(xT8[:, 0:4, :], xsrc[:, 0:4, :])
                    nc.sync.dma_start(xT8[:, 4:8, :], xsrc[:, 4:8, :])
                state[(st, "x")] = xT8

            def emit_qk_head(st):
                state[(st, "q")] = qp.tile([128, 4, ST], bf16, tag="qTb", name="qTb")
                state[(st, "k")] = qp.tile([128, 4, ST], bf16, tag="kTb", name="kTb")

            def emit_proj_qk(st, fc):
                # one 128-out-feature tile of the q/k projection + scaled phi
                xT8 = state[(st, "x")]
                qTb, kTb = state[(st, "q")], state[(st, "k")]
                pq = projp.tile([128, ST], f32, tag="proj")
                if use_fp8:
                    for dcp in range(4):
                        nc.tensor.matmul(
                            pq[:],
                            w8qk[:, 2 * dcp : 2 * dcp + 2, fc * 128 : (fc + 1) * 128],
                            xT8[:, 0, 2 * dcp : 2 * dcp + 2, :],
                            start=(dcp == 0),
                            stop=(dcp == 3),
                            perf_mode=DR,
                        )
                else:
                    for dc in range(8):
                        nc.tensor.matmul(
                            pq[:],
                            w1qk[:, dc, fc * 128 : (fc + 1) * 128],
                            xT8[:, dc, :],
                            start=(dc == 0),
                            stop=(dc == 7),
                        )
                tneg = scr.tile([128, ST], f32, tag="tneg")
                nc.scalar.activation(tneg[:], pq[:], Act.Relu, scale=-1.0 / S6)
                texp = scr.tile([128, ST], f32, tag="texp")
                nc.scalar.activation(texp[:], tneg[:], Act.Exp, scale=-1.0, bias=biasln[:])
                dst = qTb if fc < 4 else kTb
                nc.vector.scalar_tensor_tensor(
                    dst[:, fc % 4, :], pq[:], 0.0, texp[:], Alu.max, Alu.add
                )

            def emit_v_head(st):
                v1 = vp.tile([128, NCH, 8, DH + 1], bf16, tag="v1")
                nc.vector.memset(v1[:, :, :, DH], 1.0)
                state[(st, "v")] = v1

            def emit_proj_v(st, tcc):
                # one 128-token chunk of the compensated v projection
                xT8 = state[(st, "x")]
                v1 = state[(st, "v")]
                ts = slice(tcc * 128, (tcc + 1) * 128)
                pv = projp.tile([128, FPC], f32, tag="proj")
                if use_fp8:
                    for dc in range(8):
                        nc.tensor.matmul(
                            pv[:],
                            xT8[:, :, dc, ts],
                            w8v[:, dc, :, :],
                            start=(dc == 0),
                            stop=False,
                            perf_mode=DR,
                            skip_group_check=True,
                        )
                    for dcp in range(4):
                        nc.tensor.matmul(
                            pv[:],
                            xT8[:, 0, 2 * dcp : 2 * dcp + 2, ts],
                            w8vlo[:, dcp, :, :],
                            start=False,
                            stop=(dcp == 3),
                            perf_mode=DR,
                            skip_group_check=True,
                        )
                else:
                    for dc in range(8):
                        nc.tensor.matmul(
                            pv[:],
                            xT8[:, dc, ts],
                            w1v[:, dc, :],
                            start=(dc == 0),
                            stop=(dc == 7),
                        )
                nc.scalar.activation(
                    v1[:, tcc, :, 0:DH],
                    pv[:].rearrange("p (h e) -> p h e", e=DH),
                    Act.Copy,
                    scale=1.0 / S6,
                )

            def emit_chunk(st, tcc):
                # one 128-token scan chunk for all 8 heads
                gc = st * NCH + tcc
                qTb, kTb = state[(st, "q")], state[(st, "k")]
                v1 = state[(st, "v")]
                ts = slice(tcc * 128, (tcc + 1) * 128)

                # bf16 snapshot of state BEFORE this chunk (Szb persistent;
                # gc==0 uses the initial memset-zero contents)
                if gc > 0:
                    nc.scalar.copy(Szb[:], SzT[:, :, 0 : DH + 1])


                # A^T = K^T Q for 4-head groups sharing a PSUM bank
                ATp = []
                for j in range(2):
                    ATj = atp.tile([C, 4, C], f32, tag="ATp", name="ATp")
                    ATp.append(ATj)
                for h in range(8):
                    po = 64 * (h % 2)
                    fq = h // 2
                    nc.tensor.matmul(
                        ATp[h // 4][:, h % 4, :],
                        kTb[po : po + 64, fq, ts],
                        qTb[po : po + 64, fq, ts],
                        start=(h % 4 == 0),
                        stop=(h % 4 == 3),
                        skip_group_check=True,
                    )
                # k transposed to (tok, dh)
                ksb = kp.tile([128, 8, DH], bf16, tag="ksb")
                ktr = miscp.tile([128, 4, 128], bf16, tag="misc")
                for j in range(4):
                    nc.tensor.matmul(
                        ktr[:, j, :],
                        kTb[:, j, ts],
                        id128b[:],
                        is_transpose=True,
                        start=(j == 0),
                        stop=(j == 3),
                        skip_group_check=True,
                    )
                nc.vector.tensor_copy(
                    ksb[:], ktr[:].rearrange("p j (h e) -> p (j h) e", e=DH)
                )
                # causal mask + bf16 cast, one DVE op per 4 heads
                ATs = []
                for j in range(2):
                    ATsj = atsp.tile([C, 4, C], bf16, tag="ATs", name="ATs")
                    nc.vector.tensor_tensor(
                        ATsj[:], ATp[j][:], maskb[:], Alu.mult
                    )
                    ATs.append(ATsj)

                # O = A^T_m^T V' + Q S'_b  (4 heads per PSUM bank group)
                Og = []
                for j in range(2):
                    Og.append(opp.tile([C, 4, DH + 1], f32, tag="Og", name="Og"))
                for h in range(8):
                    po = 64 * (h % 2)
                    fq = h // 2
                    Oh = Og[h // 4][:, h % 4, :]
                    nc.tensor.matmul(
                        Oh,
                        ATs[h // 4][:, h % 4, :],
                        v1[:, tcc, h, :],
                        start=(h % 4 == 0),
                        stop=False,
                        skip_group_check=True,
                    )
                    nc.tensor.matmul(
                        Oh,
                        qTb[po : po + 64, fq, ts],
                        Szb[po : po + 64, h // 2, :],
                        start=False,
                        stop=(h % 4 == 3),
                        skip_group_check=True,
                    )

                # state update: accumulate K^T V'
                if use_szpsum:
                    for j in range(4):
                        nc.tensor.matmul(
                            SzT[0:64, j, 0 : DH + 1],
                            ksb[:, 2 * j, :],
                            v1[:, tcc, 2 * j, :],
                            start=(gc == 0 and j == 0),
                            stop=False,
                            skip_group_check=True,
                        )
                        nc.tensor.matmul(
                            SzT[64:128, j, 0 : DH + 1],
                            ksb[:, 2 * j + 1, :],
                            v1[:, tcc, 2 * j + 1, :],
                            start=(gc == 0 and j == 0),
                            stop=(gc == NST * NCH - 1 and j == 3),
                            skip_group_check=True,
                        )
                else:
                    Szkv = szp.tile([128, 4, DH + 1], f32, tag="Szkv", name="Szkv")
                    for j in range(4):
                        nc.tensor.matmul(
                            Szkv[0:64, j, :],
                            ksb[:, 2 * j, :],
                            v1[:, tcc, 2 * j, :],
                            start=True,
                            stop=False,
                            skip_group_check=True,
                        )
                        nc.tensor.matmul(
                            Szkv[64:128, j, :],
                            ksb[:, 2 * j + 1, :],
                            v1[:, tcc, 2 * j + 1, :],
                            start=True,
                            stop=(j == 3),
                            skip_group_check=True,
                        )
                    nc.vector.tensor_tensor(SzT[:], SzT[:], Szkv[:], Alu.add)

                # normalization: den/recip/div for 4 heads at once
                out_c = op.tile([128, 8, DH], bf16, tag="out_c")
                state[(st, "oc", tcc)] = out_c
                for j in range(2):
                    den = dp.tile([C, 4], f32, tag="den")
                    nc.vector.tensor_scalar(
                        den[:], Og[j][:, :, DH], EPS_S, None, Alu.max
                    )
                    rden = dp.tile([C, 4], f32, tag="rden")
                    nc.vector.reciprocal(rden[:], den[:])
                    nc.vector.tensor_tensor(
                        out_c[:, 4 * j : 4 * j + 4, :],
                        Og[j][:, :, 0:DH],
                        rden[:].unsqueeze(2).broadcast_to([C, 4, DH]),
                        Alu.mult,
                    )

            def emit_outtrans(st):
                outT_sb = osp.tile([128, 4, ST], bf16, tag="outT")
                state[(st, "ot")] = outT_sb
                for tcc in range(NCH):
                    oc_flat = state[(st, "oc", tcc)][:].rearrange("p h e -> p (h e)")
                    trp = miscp.tile([128, 4, C], bf16, tag="misc")
                    for g in range(4):
                        nc.tensor.matmul(
                            trp[:, g, :],
                            oc_flat[:, g * 128 : (g + 1) * 128],
                            id128b[:],
                            is_transpose=True,
                            start=(g == 0),
                            stop=(g == 3),
                            skip_group_check=True,
                        )
                    nc.vector.tensor_copy(
                        outT_sb[:, :, tcc * 128 : (tcc + 1) * 128], trp[:]
                    )

            def emit_o3_head(st):
                state[(st, "o3")] = o3p.tile([128, 8, ST], bf16, tag="o3sb", name="o3sb")

            def emit_outproj(st, oc):
                outT_sb = state[(st, "ot")]
                o3sb = state[(st, "o3")]
                po3 = projp.tile([128, ST], f32, tag="proj")
                for g in range(4):
                    nc.tensor.matmul(
                        po3[:],
                        w2[:, g, oc * 128 : (oc + 1) * 128],
                        outT_sb[:, g, :],
                        start=(g == 0),
                        stop=(g == 3),
                    )
                if oc % 2 == 0:
                    nc.scalar.copy(o3sb[:, oc, :], po3[:])
                else:
                    nc.vector.tensor_copy(o3sb[:, oc, :], po3[:])

            def emit_outdma(st, half):
                t0 = st * ST
                o3sb = state[(st, "o3")]
                dst = outT_d[:, t0 : t0 + ST].rearrange("(o p) t -> p o t", p=128)
                eng = nc.gpsimd if odma_gp else nc.sync
                eng.dma_start(
                    dst[:, 4 * half : 4 * half + 4, :],
                    o3sb[:, 4 * half : 4 * half + 4, :],
                )

            # ---- software-pipelined emission: proj(st) interleaves with
            # ---- attention/output of st-1
            emit_xdma(0)
            for st in range(NST + 1):
                cur = st < NST
                prv = st > 0
                if cur:
                    if st + 1 < NST:
                        emit_xdma(st + 1)
                    emit_qk_head(st)
                    emit_proj_qk(st, 0)
                    emit_proj_qk(st, 1)
                if prv:
                    emit_chunk(st - 1, 0)
                if cur:
                    emit_proj_qk(st, 2)
                    emit_proj_qk(st, 3)
                if prv:
                    emit_chunk(st - 1, 1)
                if cur:
                    emit_proj_qk(st, 4)
                    emit_proj_qk(st, 5)
                if prv:
                    emit_chunk(st - 1, 2)
                if cur:
                    emit_proj_qk(st, 6)
                    emit_proj_qk(st, 7)
                if prv:
                    emit_chunk(st - 1, 3)
                if cur:
                    emit_v_head(st)
                    emit_proj_v(st, 0)
                    emit_proj_v(st, 1)
                if prv:
                    emit_outtrans(st - 1)
                    emit_o3_head(st - 1)
                if cur:
                    emit_proj_v(st, 2)
                if prv:
                    emit_outproj(st - 1, 0)
                    emit_outproj(st - 1, 1)
                    emit_outproj(st - 1, 2)
                    emit_outproj(st - 1, 3)
                if cur:
                    emit_proj_v(st, 3)
                if prv:
                    emit_outdma(st - 1, 0)
                    emit_outproj(st - 1, 4)
                    emit_outproj(st - 1, 5)
                    emit_outproj(st - 1, 6)
                    emit_outproj(st - 1, 7)
                    emit_outdma(st - 1, 1)

    nc.finalize()
    return nc


def _get_nc():
    key = (USE_FP8, USE_SZPSUM, ODMA_GP)
    if key not in _CACHE:
        _CACHE[key] = _build_nc()
    return _CACHE[key]


def _make_in_maps(x, w_qkv, w_out):
    E4 = ml_dtypes.float8_e4m3
    maskb = np.broadcast_to(
        np.triu(np.ones((C, C), np.float32))[:, None, :], (C, 4, C)
    ).astype(ml_dtypes.bfloat16)
    id128b = np.eye(128, dtype=np.float32).astype(ml_dtypes.bfloat16)
    in_maps = []
    for core in range(8):
        b, g = core // 2, core % 2
        xT = np.ascontiguousarray(x[b].T)
        wqk = np.ascontiguousarray(np.concatenate(
            [
                w_qkv[512 * g : 512 * (g + 1)],
                w_qkv[1024 + 512 * g : 1024 + 512 * (g + 1)],
            ],
            axis=0,
        ).T) * S6
        wv = np.ascontiguousarray(w_qkv[2048 + 512 * g : 2048 + 512 * (g + 1)].T) * S6
        w2 = np.ascontiguousarray(w_out[:, 512 * g : 512 * (g + 1)].T).astype(
            ml_dtypes.bfloat16
        )
        m = {"w2": w2, "maskb": maskb, "id128b": id128b}
        if USE_FP8:
            xhi = xT.astype(E4)
            m["xhi"] = xhi
            m["xlo"] = (xT - xhi.astype(np.float32)).astype(E4)
            m["w8qk"] = wqk.astype(E4)
            wv_hi = wv.astype(E4)
            m["w8v"] = np.ascontiguousarray(np.stack([wv_hi, wv_hi], axis=1))
            m["w8vlo"] = (wv - wv_hi.astype(np.float32)).astype(E4)
        else:
            m["xT"] = xT
            m["w1qk"] = wqk.astype(np.float32)
            m["w1v"] = wv.astype(np.float32)
        in_maps.append(m)
    return in_maps


def _run(inputs, trace=False):
    from concourse.bass_utils import run_bass_kernel_spmd

    nc = _get_nc()
    in_maps = _make_in_maps(inputs["x"], inputs["w_qkv"], inputs["w_out"])
    res = run_bass_kernel_spmd(nc, in_maps, core_ids=list(range(8)), trace=trace)
    out = np.empty((B, L, D), np.float32)
    for b in range(B):
        p0 = res.results[2 * b]["outT"].astype(np.float32)
        p1 = res.results[2 * b + 1]["outT"].astype(np.float32)
        out[b] = (p0 + p1).T
    return out, res


def kernel(x, w_qkv, w_out):
    out, _ = _run({"x": x, "w_qkv": w_qkv, "w_out": w_out})
    return out


# revision 17
# speedup vs baseline: 1.1415x; 1.0104x over previous
"""TRN2 Bass kernel for nn_LinearAttention (B=4, L=4096, D=1024, H=16, dh=64).

Sharding: core c handles batch c//2, heads (c%2)*8..+8. Zero cross-core
communication; the two half-head partial output projections per batch are
summed on the host during unshard (partials DMA'd in bf16).

Per-core pipeline (super-tiles of 512 tokens, scan chunks of 128), with the
projections in fp8 DoubleRow (0.5 cycles/row, 256-contraction per matmul):
  - q/k proj: x and w both plain e4m3 (w pre-scaled by 2^6 to clear the
    e4m3 subnormal range).  Scaled-phi trick: phi_s = max(pq,0) + 64*exp(
    min(pq/64,0)) = 64*phi(x); the 2^6 per-side scale cancels in num/den
    (EPS scaled by 2^12).  q/k quantization error is damped by the
    normalization ratio (measured end-to-end ~2.8e-3).
  - v proj: error-compensated fp8: x = x_hi + x_lo (two e4m3 planes,
    host-split), w row-pairs (w_hi, w_lo): pv = (x_hi+x_lo)W_hi + x_hi W_lo
    (12 DoubleRow matmuls per 128-token chunk).  v1 copy descales by 2^-6.
  - attention: bf16 chunked scan, A^T = K^T Q masked (4 heads batched per
    PSUM bank, one DVE mask op per 4 heads), O = A^T_m^T V' + Q S'_b.
    State S' accumulates IN PSUM across all 32 chunks (matmul accumulation,
    no DVE adds); a bf16 copy S'_b is taken each chunk for the Q S' matmul
    (moving operand must be non-f32 so the matmul runs at 1 cycle/row).
  - out proj: w2 f32r stationary x bf16 moving (1 cycle/row), bf16 partial
    written to HBM (halves output DMA traffic).
Software-pipelined emission: projection of super-tile st interleaves with
attention/output of st-1 so the PE is never hostage to the phi (Act) chain.
End-to-end rel err vs fp32 reference: ~2.8e-3.
"""
import sys

sys.path.insert(0, "/opt/trn_rl_repo")
import numpy as np
import ml_dtypes

D = 1024
L = 4096
B = 4
H = 16
DH = 64
FPC = 512          # features per core (8 heads x 64)
C = 128            # scan chunk
ST = 512           # super-tile tokens
NCH = ST // C      # 4
NST = L // ST      # 8
S6 = 64.0          # 2^6 weight pre-scale
EPS_S = 1e-6 * S6 * S6
LN_S6 = float(np.log(S6))

_CACHE = {}
import os
USE_FP8 = os.environ.get("K_FP8", "1") == "1"
USE_SZPSUM = os.environ.get("K_SZPSUM", "1") == "1"
ODMA_GP = os.environ.get("K_ODMA", "gp") == "gp"
USE_ILV = os.environ.get("K_ILV", "1") == "1"


def _build_nc(use_fp8=None, use_szpsum=None):
    use_fp8 = USE_FP8 if use_fp8 is None else use_fp8
    use_szpsum = USE_SZPSUM if use_szpsum is None else use_szpsum
    odma_gp = ODMA_GP
    import concourse.bacc as bacc
    import concourse.mybir as mybir
    import concourse.tile as tile

    dt = mybir.dt
    f32, f32r, bf16, f8 = dt.float32, dt.float32r, dt.bfloat16, dt.float8e4
    Alu = mybir.AluOpType
    Act = mybir.ActivationFunctionType
    DR = mybir.MatmulPerfMode.DoubleRow

    nc = bacc.Bacc("TRN2", target_bir_lowering=False, debug=True)

    if use_fp8:
        xhi_d = nc.dram_tensor("xhi", [D, L], f8, kind="ExternalInput")
        xlo_d = nc.dram_tensor("xlo", [D, L], f8, kind="ExternalInput")
        w8qk_d = nc.dram_tensor("w8qk", [D, 2 * FPC], f8, kind="ExternalInput")
        w8v_d = nc.dram_tensor("w8v", [D, 2, FPC], f8, kind="ExternalInput")
        w8vlo_d = nc.dram_tensor("w8vlo", [D, FPC], f8, kind="ExternalInput")
    else:
        xT_d = nc.dram_tensor("xT", [D, L], f32r, kind="ExternalInput")
        w1qk_d = nc.dram_tensor("w1qk", [D, 2 * FPC], f32r, kind="ExternalInput")
        w1v_d = nc.dram_tensor("w1v", [D, FPC], f32r, kind="ExternalInput")
    w2_d = nc.dram_tensor("w2", [FPC, D], bf16, kind="ExternalInput")
    maskb_d = nc.dram_tensor("maskb", [C, 4, C], bf16, kind="ExternalInput")
    id128b_d = nc.dram_tensor("id128b", [128, 128], bf16, kind="ExternalInput")
    outT_d = nc.dram_tensor("outT", [D, L], bf16, kind="ExternalOutput")

    with tile.TileContext(nc) as tc:
        with (
            tc.tile_pool(name="wp", bufs=1) as wp,
            tc.tile_pool(name="xp", bufs=2) as xp,
            tc.tile_pool(name="qp", bufs=2) as qp,
            tc.tile_pool(name="vp", bufs=2) as vp,
            tc.tile_pool(name="scr", bufs=3) as scr,
            tc.tile_pool(name="atsp", bufs=3) as atsp,
            tc.tile_pool(name="kp", bufs=2) as kp,
            tc.tile_pool(name="dp", bufs=4) as dp,
            tc.tile_pool(name="op", bufs=5) as op,
            tc.tile_pool(name="osp", bufs=2) as osp,
            tc.tile_pool(name="o3p", bufs=2) as o3p,
            tc.tile_pool(name="szbp", bufs=1) as szbp,
            tc.tile_pool(name="projp", bufs=2, space="PSUM") as projp,
            tc.tile_pool(name="atp", bufs=2, space="PSUM") as atp,
            tc.tile_pool(name="opp", bufs=2, space="PSUM") as opp,
            tc.tile_pool(name="miscp", bufs=1, space="PSUM") as miscp,
            tc.tile_pool(name="szp", bufs=1, space="PSUM") as szp,
            tc.tile_pool(name="szsb", bufs=1) as szsb,
        ):
            # ---- resident weights/constants, spread across DMA queues ----
            if use_fp8:
                w8qk = wp.tile([128, 8, 2 * FPC], f8, tag="w8qk")
                wqk_src = w8qk_d[:].rearrange("(c p) f -> p c f", p=128)
                nc.sync.dma_start(w8qk[:, :, 0:FPC], wqk_src[:, :, 0:FPC])
                xhi0 = xp.tile([128, 2, 8, ST], f8, tag="xT8")
                nc.scalar.dma_start(
                    xhi0[:, 0], xhi_d[:, 0:ST].rearrange("(c p) t -> p c t", p=128)
                )
                nc.sync.dma_start(
                    w8qk[:, :, FPC : 2 * FPC], wqk_src[:, :, FPC : 2 * FPC]
                )
                nc.scalar.dma_start(
                    xhi0[:, 1], xlo_d[:, 0:ST].rearrange("(c p) t -> p c t", p=128)
                )
                w8v = wp.tile([128, 8, 2, FPC], f8, tag="w8v")
                nc.scalar.dma_start(
                    w8v[:], w8v_d[:].rearrange("(c p) two f -> p c two f", p=128)
                )
                w8vlo = wp.tile([128, 4, 2, FPC], f8, tag="w8vlo")
                nc.scalar.dma_start(
                    w8vlo[:],
                    w8vlo_d[:].rearrange("(dcp two p) f -> p dcp two f", p=128, two=2),
                )
            else:
                w1qk = wp.tile([128, 8, 2 * FPC], f32r, tag="w1qk")
                wqk_src = w1qk_d[:].rearrange("(c p) f -> p c f", p=128)
                nc.sync.dma_start(w1qk[:, :, 0:FPC], wqk_src[:, :, 0:FPC])
                xhi0 = xp.tile([128, 8, ST], f32r, tag="xT8")
                nc.scalar.dma_start(
                    xhi0[:, 0:4, :],
                    xT_d[:, 0:ST].rearrange("(c p) t -> p c t", p=128)[:, 0:4, :],
                )
                nc.sync.dma_start(
                    w1qk[:, :, FPC : 2 * FPC], wqk_src[:, :, FPC : 2 * FPC]
                )
                nc.scalar.dma_start(
                    xhi0[:, 4:8, :],
                    xT_d[:, 0:ST].rearrange("(c p) t -> p c t", p=128)[:, 4:8, :],
                )
                w1v = wp.tile([128, 8, FPC], f32r, tag="w1v")
                nc.scalar.dma_start(
                    w1v[:], w1v_d[:].rearrange("(c p) f -> p c f", p=128)
                )
            maskb = wp.tile([C, 4, C], bf16, tag="maskb")
            nc.gpsimd.dma_start(maskb[:], maskb_d[:])
            id128b = wp.tile([128, 128], bf16, tag="id128b")
            nc.gpsimd.dma_start(id128b[:], id128b_d[:])
            w2 = wp.tile([128, 4, D], bf16, tag="w2")
            nc.gpsimd.dma_start(w2[:], w2_d[:].rearrange("(g p) f -> p g f", p=128))

            biasln = wp.tile([128, 1], f32, tag="biasln")
            nc.vector.memset(biasln[:], LN_S6)

            # persistent scan state: accumulated in PSUM across all chunks
            if use_szpsum:
                # padded to 2048B/partition so [64:128] slice offsets stay
                # aligned with the executor's 2KB pending-zero rows
                SzT = szp.tile([128, 4, 2 * DH], f32, tag="Sz", name="Sz")
                SzW = 2 * DH
            else:
                SzT = szsb.tile([128, 4, DH + 1], f32, tag="Sz", name="Sz")
                SzW = DH + 1
                nc.vector.memset(SzT[:], 0.0)
            # bf16 state snapshot (moving operand of the Q.S matmul)
            Szb = szbp.tile([128, 4, DH + 1], bf16, tag="Szb")
            nc.vector.memset(Szb[:], 0.0)

            state = {}

            def emit_xdma(st):
                t0 = st * ST
                if st == 0:
                    xT8 = xhi0
                elif use_fp8:
                    xT8 = xp.tile([128, 2, 8, ST], f8, tag="xT8")
                    nc.sync.dma_start(
                        xT8[:, 0],
                        xhi_d[:, t0 : t0 + ST].rearrange("(c p) t -> p c t", p=128),
                    )
                    nc.sync.dma_start(
                        xT8[:, 1],
                        xlo_d[:, t0 : t0 + ST].rearrange("(c p) t -> p c t", p=128),
                    )
                else:
                    xT8 = xp.tile([128, 8, ST], f32r, tag="xT8")
                    xsrc = xT_d[:, t0 : t0 + ST].rearrange("(c p) t -> p c t", p=128)
                    nc.sync.dma_start(xT8[:, 0:4, :], xsrc[:, 0:4, :])
                    nc.sync.dma_start(xT8[:, 4:8, :], xsrc[:, 4:8, :])
                state[(st, "x")] = xT8

            def emit_qk_head(st):
                state[(st, "q")] = qp.tile([128, 4, ST], bf16, tag="qTb", name="qTb")
                state[(st, "k")] = qp.tile([128, 4, ST], bf16, tag="kTb", name="kTb")

            def emit_proj_qk(st, fc):
                # one 128-out-feature tile of the q/k projection + scaled phi
                xT8 = state[(st, "x")]
                qTb, kTb = state[(st, "q")], state[(st, "k")]
                pq = projp.tile([128, ST], f32, tag="proj")
                if use_fp8:
                    for dcp in range(4):
                        nc.tensor.matmul(
                            pq[:],
                            w8qk[:, 2 * dcp : 2 * dcp + 2, fc * 128 : (fc + 1) * 128],
                            xT8[:, 0, 2 * dcp : 2 * dcp + 2, :],
                            start=(dcp == 0),
                            stop=(dcp == 3),
                            perf_mode=DR,
                        )
                else:
                    for dc in range(8):
                        nc.tensor.matmul(
                            pq[:],
                            w1qk[:, dc, fc * 128 : (fc + 1) * 128],
                            xT8[:, dc, :],
                            start=(dc == 0),
                            stop=(dc == 7),
                        )
                tneg = scr.tile([128, ST], f32, tag="tneg")
                nc.scalar.activation(tneg[:], pq[:], Act.Relu, scale=-1.0 / S6)
                texp = scr.tile([128, ST], f32, tag="texp")
                nc.scalar.activation(texp[:], tneg[:], Act.Exp, scale=-1.0, bias=biasln[:])
                dst = qTb if fc < 4 else kTb
                nc.vector.scalar_tensor_tensor(
                    dst[:, fc % 4, :], pq[:], 0.0, texp[:], Alu.max, Alu.add
                )

            def emit_v_head(st):
                v1 = vp.tile([128, NCH, 8, DH + 1], bf16, tag="v1")
                nc.vector.memset(v1[:, :, :, DH], 1.0)
                state[(st, "v")] = v1

            def emit_proj_v(st, tcc):
                # one 128-token chunk of the compensated v projection
                xT8 = state[(st, "x")]
                v1 = state[(st, "v")]
                ts = slice(tcc * 128, (tcc + 1) * 128)
                pv = projp.tile([128, FPC], f32, tag="proj")
                if use_fp8:
                    for dc in range(8):
                        nc.tensor.matmul(
                            pv[:],
                            xT8[:, :, dc, ts],
                            w8v[:, dc, :, :],
                            start=(dc == 0),
                            stop=False,
                            perf_mode=DR,
                            skip_group_check=True,
                        )
                    for dcp in range(4):
                        nc.tensor.matmul(
                            pv[:],
                            xT8[:, 0, 2 * dcp : 2 * dcp + 2, ts],
                            w8vlo[:, dcp, :, :],
                            start=False,
                            stop=(dcp == 3),
                            perf_mode=DR,
                            skip_group_check=True,
                        )
                else:
                    for dc in range(8):
                        nc.tensor.matmul(
                            pv[:],
                            xT8[:, dc, ts],
                            w1v[:, dc, :],
                            start=(dc == 0),
                            stop=(dc == 7),
                        )
                nc.scalar.activation(
                    v1[:, tcc, :, 0:DH],
                    pv[:].rearrange("p (h e) -> p h e", e=DH),
                    Act.Copy,
                    scale=1.0 / S6,
                )

            def emit_chunk(st, tcc):
                # one 128-token scan chunk for all 8 heads
                gc = st * NCH + tcc
                qTb, kTb = state[(st, "q")], state[(st, "k")]
                v1 = state[(st, "v")]
                ts = slice(tcc * 128, (tcc + 1) * 128)

                # bf16 snapshot of state BEFORE this chunk (Szb persistent;
                # gc==0 uses the initial memset-zero contents)
                if gc > 0:
                    nc.scalar.copy(Szb[:], SzT[:, :, 0 : DH + 1])


                # A^T = K^T Q for 4-head groups sharing a PSUM bank
                ATp = []
                for j in range(2):
                    ATj = atp.tile([C, 4, C], f32, tag="ATp", name="ATp")
                    ATp.append(ATj)
                for h in range(8):
                    po = 64 * (h % 2)
                    fq = h // 2
                    nc.tensor.matmul(
                        ATp[h // 4][:, h % 4, :],
                        kTb[po : po + 64, fq, ts],
                        qTb[po : po + 64, fq, ts],
                        start=(h % 4 == 0),
                        stop=(h % 4 == 3),
                        skip_group_check=True,
                    )
                # k transposed to (tok, dh)
                ksb = kp.tile([128, 8, DH], bf16, tag="ksb")
                ktr = miscp.tile([128, 4, 128], bf16, tag="misc")
                for j in range(4):
                    nc.tensor.matmul(
                        ktr[:, j, :],
                        kTb[:, j, ts],
                        id128b[:],
                        is_transpose=True,
                        start=(j == 0),
                        stop=(j == 3),
                        skip_group_check=True,
                    )
                nc.vector.tensor_copy(
                    ksb[:], ktr[:].rearrange("p j (h e) -> p (j h) e", e=DH)
                )
                # causal mask + bf16 cast, one DVE op per 4 heads
                ATs = []
                for j in range(2):
                    ATsj = atsp.tile([C, 4, C], bf16, tag="ATs", name="ATs")
                    nc.vector.tensor_tensor(
                        ATsj[:], ATp[j][:], maskb[:], Alu.mult
                    )
                    ATs.append(ATsj)

                # O = A^T_m^T V' + Q S'_b  (4 heads per PSUM bank group)
                Og = []
                for j in range(2):
                    Og.append(opp.tile([C, 4, DH + 1], f32, tag="Og", name="Og"))
                for h in range(8):
                    po = 64 * (h % 2)
                    fq = h // 2
                    Oh = Og[h // 4][:, h % 4, :]
                    nc.tensor.matmul(
                        Oh,
                        ATs[h // 4][:, h % 4, :],
                        v1[:, tcc, h, :],
                        start=(h % 4 == 0),
                        stop=False,
                        skip_group_check=True,
                    )
                    nc.tensor.matmul(
                        Oh,
                        qTb[po : po + 64, fq, ts],
                        Szb[po : po + 64, h // 2, :],
                        start=False,
                        stop=(h % 4 == 3),
                        skip_group_check=True,
                    )

                # state update: accumulate K^T V'
                if use_szpsum:
                    for j in range(4):
                        nc.tensor.matmul(
                            SzT[0:64, j, 0 : DH + 1],
                            ksb[:, 2 * j, :],
                            v1[:, tcc, 2 * j, :],
                            start=(gc == 0 and j == 0),
                            stop=False,
                            skip_group_check=True,
                        )
                        nc.tensor.matmul(
                            SzT[64:128, j, 0 : DH + 1],
                            ksb[:, 2 * j + 1, :],
                            v1[:, tcc, 2 * j + 1, :],
                            start=(gc == 0 and j == 0),
                            stop=(gc == NST * NCH - 1 and j == 3),
                            skip_group_check=True,
                        )
                else:
                    Szkv = szp.tile([128, 4, DH + 1], f32, tag="Szkv", name="Szkv")
                    for j in range(4):
                        nc.tensor.matmul(
                            Szkv[0:64, j, :],
                            ksb[:, 2 * j, :],
                            v1[:, tcc, 2 * j, :],
                            start=True,
                            stop=False,
                            skip_group_check=True,
                        )
                        nc.tensor.matmul(
                            Szkv[64:128, j, :],
                            ksb[:, 2 * j + 1, :],
                            v1[:, tcc, 2 * j + 1, :],
                            start=True,
                            stop=(j == 3),
                            skip_group_check=True,
                        )
                    nc.vector.tensor_tensor(SzT[:], SzT[:], Szkv[:], Alu.add)

                # normalization: den/recip/div for 4 heads at once
                out_c = op.tile([128, 8, DH], bf16, tag="out_c")
                state[(st, "oc", tcc)] = out_c
                for j in range(2):
                    den = dp.tile([C, 4], f32, tag="den")
                    nc.vector.tensor_scalar(
                        den[:], Og[j][:, :, DH], EPS_S, None, Alu.max
                    )
                    rden = dp.tile([C, 4], f32, tag="rden")
                    nc.vector.reciprocal(rden[:], den[:])
                    nc.vector.tensor_tensor(
                        out_c[:, 4 * j : 4 * j + 4, :],
                        Og[j][:, :, 0:DH],
                        rden[:].unsqueeze(2).broadcast_to([C, 4, DH]),
                        Alu.mult,
                    )

            def emit_outtrans(st):
                outT_sb = osp.tile([128, 4, ST], bf16, tag="outT")
                state[(st, "ot")] = outT_sb
                for tcc in range(NCH):
                    oc_flat = state[(st, "oc", tcc)][:].rearrange("p h e -> p (h e)")
                    trp = miscp.tile([128, 4, C], bf16, tag="misc")
                    for g in range(4):
                        nc.tensor.matmul(
                            trp[:, g, :],
                            oc_flat[:, g * 128 : (g + 1) * 128],
                            id128b[:],
                            is_transpose=True,
                            start=(g == 0),
                            stop=(g == 3),
                            skip_group_check=True,
                        )
                    nc.vector.tensor_copy(
                        outT_sb[:, :, tcc * 128 : (tcc + 1) * 128], trp[:]
                    )

            def emit_o3_head(st):
                state[(st, "o3")] = o3p.tile([128, 8, ST], bf16, tag="o3sb", name="o3sb")

            def emit_outproj(st, oc):
                outT_sb = state[(st, "ot")]
                o3sb = state[(st, "o3")]
                po3 = projp.tile([128, ST], f32, tag="proj")
                for g in range(4):
                    nc.tensor.matmul(
                        po3[:],
                        w2[:, g, oc * 128 : (oc + 1) * 128],
                        outT_sb[:, g, :],
                        start=(g == 0),
                        stop=(g == 3),
                    )
                if oc % 2 == 0:
                    nc.scalar.copy(o3sb[:, oc, :], po3[:])
                else:
                    nc.vector.tensor_copy(o3sb[:, oc, :], po3[:])

            def emit_outdma(st, half):
                t0 = st * ST
                o3sb = state[(st, "o3")]
                dst = outT_d[:, t0 : t0 + ST].rearrange("(o p) t -> p o t", p=128)
                eng = nc.gpsimd if odma_gp else nc.sync
                eng.dma_start(
                    dst[:, 4 * half : 4 * half + 4, :],
                    o3sb[:, 4 * half : 4 * half + 4, :],
                )

            # ---- software-pipelined emission: proj(st) interleaves with
            # ---- attention/output of st-1
            emit_xdma(0)
            if not USE_ILV:
                for st in range(NST):
                    if st + 1 < NST:
                        emit_xdma(st + 1)
                    emit_qk_head(st)
                    for fc in range(8):
                        emit_proj_qk(st, fc)
                    emit_v_head(st)
                    for tcc in range(NCH):
                        emit_proj_v(st, tcc)
                    for tcc in range(NCH):
                        emit_chunk(st, tcc)
                    emit_outtrans(st)
                    emit_o3_head(st)
                    for oc in range(4):
                        emit_outproj(st, oc)
                    emit_outdma(st, 0)
                    for oc in range(4, 8):
                        emit_outproj(st, oc)
                    emit_outdma(st, 1)
            for st in range(NST + 1) if USE_ILV else []:
                cur = st < NST
                prv = st > 0
                if cur:
                    if st + 1 < NST:
                        emit_xdma(st + 1)
                    emit_qk_head(st)
                    emit_proj_qk(st, 0)
                    emit_proj_qk(st, 1)
                if prv:
                    emit_chunk(st - 1, 0)
                if cur:
                    emit_proj_qk(st, 2)
                    emit_proj_qk(st, 3)
                if prv:
                    emit_chunk(st - 1, 1)
                if cur:
                    emit_proj_qk(st, 4)
                    emit_proj_qk(st, 5)
                if prv:
                    emit_chunk(st - 1, 2)
                if cur:
                    emit_proj_qk(st, 6)
                    emit_proj_qk(st, 7)
                if prv:
                    emit_chunk(st - 1, 3)
                if cur:
                    emit_v_head(st)
                    emit_proj_v(st, 0)
                    emit_proj_v(st, 1)
                if prv:
                    emit_outtrans(st - 1)
                    emit_o3_head(st - 1)
                if cur:
                    emit_proj_v(st, 2)
                if prv:
                    emit_outproj(st - 1, 0)
                    emit_outproj(st - 1, 1)
                    emit_outproj(st - 1, 2)
                    emit_outproj(st - 1, 3)
                if cur:
                    emit_proj_v(st, 3)
                if prv:
                    emit_outdma(st - 1, 0)
                    emit_outproj(st - 1, 4)
                    emit_outproj(st - 1, 5)
                    emit_outproj(st - 1, 6)
                    emit_outproj(st - 1, 7)
                    emit_outdma(st - 1, 1)

    nc.finalize()
    return nc


def _get_nc():
    key = (USE_FP8, USE_SZPSUM, ODMA_GP, USE_ILV)
    if key not in _CACHE:
        _CACHE[key] = _build_nc()
    return _CACHE[key]


def _make_in_maps(x, w_qkv, w_out):
    E4 = ml_dtypes.float8_e4m3
    maskb = np.broadcast_to(
        np.triu(np.ones((C, C), np.float32))[:, None, :], (C, 4, C)
    ).astype(ml_dtypes.bfloat16)
    id128b = np.eye(128, dtype=np.float32).astype(ml_dtypes.bfloat16)
    in_maps = []
    for core in range(8):
        b, g = core // 2, core % 2
        xT = np.ascontiguousarray(x[b].T)
        wqk = np.ascontiguousarray(np.concatenate(
            [
                w_qkv[512 * g : 512 * (g + 1)],
                w_qkv[1024 + 512 * g : 1024 + 512 * (g + 1)],
            ],
            axis=0,
        ).T) * S6
        wv = np.ascontiguousarray(w_qkv[2048 + 512 * g : 2048 + 512 * (g + 1)].T) * S6
        w2 = np.ascontiguousarray(w_out[:, 512 * g : 512 * (g + 1)].T).astype(
            ml_dtypes.bfloat16
        )
        m = {"w2": w2, "maskb": maskb, "id128b": id128b}
        if USE_FP8:
            xhi = xT.astype(E4)
            m["xhi"] = xhi
            m["xlo"] = (xT - xhi.astype(np.float32)).astype(E4)
            m["w8qk"] = wqk.astype(E4)
            wv_hi = wv.astype(E4)
            m["w8v"] = np.ascontiguousarray(np.stack([wv_hi, wv_hi], axis=1))
            m["w8vlo"] = (wv - wv_hi.astype(np.float32)).astype(E4)
        else:
            m["xT"] = xT
            m["w1qk"] = wqk.astype(np.float32)
            m["w1v"] = wv.astype(np.float32)
        in_maps.append(m)
    return in_maps


def _run(inputs, trace=False):
    from concourse.bass_utils import run_bass_kernel_spmd

    nc = _get_nc()
    in_maps = _make_in_maps(inputs["x"], inputs["w_qkv"], inputs["w_out"])
    res = run_bass_kernel_spmd(nc, in_maps, core_ids=list(range(8)), trace=trace)
    out = np.empty((B, L, D), np.float32)
    for b in range(B):
        p0 = res.results[2 * b]["outT"].astype(np.float32)
        p1 = res.results[2 * b + 1]["outT"].astype(np.float32)
        out[b] = (p0 + p1).T
    return out, res


def kernel(x, w_qkv, w_out):
    out, _ = _run({"x": x, "w_qkv": w_qkv, "w_out": w_out})
    return out
